# revision 1
# baseline (speedup 1.0000x reference)
"""Trainium2 Bass kernel for multi-grid trilinear feature sampling
(AMG encoder: 64 grids x [2, 64,64,64], 262144 query points, affine
per-grid transforms, grid_sample trilinear / align_corners=True /
zeros padding).

Strategy (grid-parallel over 8 NeuronCores, 8 grids per core):
- Host: per grid, build a corner-dilated table R[cell, 16] where cell
  enumerates the 65^3 sampling cells of the zero-padded grid and the 16
  values are the cell's 8 corner voxels x 2 features (order f*8+dz*4+
  dy*2+dx). One 64B gather row yields everything a sample needs.
- Device: PE computes s = px+0.5 for all 8 grids (affine fold of the
  coordinate normalization into the matrices), DVE derives cell indices
  (round-to-nearest of s-0.0 with the +-0.5 trick), trilinear corner
  weights and validity masks, gpsimd indirect-DMA gathers one 64B row
  per (point, grid) sample, DVE does the weighted reduction.
- Output per core: [N, 16] slab (8 grids x 2 features); host concats.
"""
import sys

sys.path.insert(0, "/opt/trn_rl_repo")
import numpy as np

import concourse.bass as bass
import concourse.mybir as mybir
from concourse import bacc
from concourse.tile import TileContext
from concourse.bass_utils import run_bass_kernel_spmd

P = 128
N_POINTS = 262144
N_GRIDS = 64
N_CORES = 8
GPC = N_GRIDS // N_CORES      # grids per core = 8
NF = 2
GS = 64                       # grid side
NC_CELL = GS + 1              # 65 cells per axis
NPAD = GS + 2                 # 66 padded grid points per axis
CELLS = NC_CELL ** 3          # 274625 cells per grid
T_ROWS = GPC * CELLS          # table rows per core

SUPER = 4096                  # points per supertile
NB = SUPER // P               # batches of 128 points per supertile = 32
N_ITERS = N_POINTS // SUPER   # 64

F32 = mybir.dt.float32
I32 = mybir.dt.int32
AX = mybir.AluOpType

CLAMP_HI = np.float32(64.49997)


def build_kernel(n_iters=N_ITERS, unroll=4):
    nc = bacc.Bacc("TRN2", target_bir_lowering=False)
    table = nc.dram_tensor("table", [T_ROWS, 16], F32, kind="ExternalInput")
    xt4 = nc.dram_tensor("xt4", [4, N_POINTS], F32, kind="ExternalInput")
    mpack = nc.dram_tensor("mpack", [4, 24], F32, kind="ExternalInput")
    cbase = nc.dram_tensor("cbase", [P, GPC], F32, kind="ExternalInput")
    out = nc.dram_tensor("out", [N_POINTS, 16], F32, kind="ExternalOutput")
    out_v = out[:].rearrange("(s b p) e -> s b p e", b=NB, p=P)

    with TileContext(nc) as tc:
        with tc.tile_pool(name="const", bufs=1) as cpool, \
             tc.tile_pool(name="sbuf", bufs=2) as pool, \
             tc.tile_pool(name="psum", bufs=2, space="PSUM") as psum:
            mp_sb = cpool.tile([4, 24], F32)
            nc.sync.dma_start(mp_sb[:], mpack[:])
            cb_sb = cpool.tile([P, GPC], F32)
            nc.sync.dma_start(cb_sb[:], cbase[:])

            def body(i):
                xt = pool.tile([4, SUPER], F32, tag="xt")
                nc.sync.dma_start(xt[:], xt4[:, bass.ts(i, SUPER)])

                s_ps = psum.tile([P, NB, 32], F32, space="PSUM", tag="sps")
                for b in range(NB):
                    nc.tensor.matmul(
                        s_ps[:, b, :24], lhsT=xt[:, bass.ts(b, P)], rhs=mp_sb[:],
                        start=True, stop=True)
                s = pool.tile([P, NB, 24], F32, tag="s")
                nc.scalar.copy(s[:], s_ps[:, :, :24])

                # masks from raw s: valid iff -0.5 <= s < 64.5 per axis
                mlo = pool.tile([P, NB, 24], F32, tag="mlo")
                nc.vector.tensor_scalar(mlo[:], s[:], -0.5, None, op0=AX.is_ge)
                mhi = pool.tile([P, NB, 24], F32, tag="mhi")
                nc.vector.tensor_scalar(mhi[:], s[:], 64.5, None, op0=AX.is_lt)
                mm = mlo
                nc.vector.tensor_tensor(mm[:], mlo[:], mhi[:], op=AX.mult)
                mm4 = mm[:].rearrange("p b (g a) -> p b g a", a=3)
                m3 = pool.tile([P, NB, GPC], F32, tag="m3")
                nc.vector.tensor_tensor(m3[:], mm4[:, :, :, 0], mm4[:, :, :, 1], op=AX.mult)
                nc.vector.tensor_tensor(m3[:], m3[:], mm4[:, :, :, 2], op=AX.mult)

                # u = clamp(s, 0, 64.49997); celli = rne(u); d = u - cellf
                u = pool.tile([P, NB, 24], F32, tag="u")
                nc.vector.tensor_scalar(u[:], s[:], -0.5, float(CLAMP_HI),
                                        op0=AX.max, op1=AX.min)
                celli = pool.tile([P, NB, 24], I32, tag="celli")
                nc.vector.tensor_copy(celli[:], u[:])
                cellf = pool.tile([P, NB, 24], F32, tag="cellf")
                nc.vector.tensor_copy(cellf[:], celli[:])
                d = u
                nc.vector.tensor_tensor(d[:], u[:], cellf[:], op=AX.subtract)
                d4 = d[:].rearrange("p b (g a) -> p b g a", a=3)
                c4 = cellf[:].rearrange("p b (g a) -> p b g a", a=3)

                # weight pairs per axis: w1 = d + 0.5, w0 = 0.5 - d
                wx = pool.tile([P, NB, GPC, 2], F32, tag="wx")
                nc.vector.tensor_scalar(wx[:, :, :, 1], d4[:, :, :, 0], 1.0, 0.5,
                                        op0=AX.mult, op1=AX.add)
                nc.vector.tensor_scalar(wx[:, :, :, 0], d4[:, :, :, 0], -1.0, 0.5,
                                        op0=AX.mult, op1=AX.add)
                wy = pool.tile([P, NB, GPC, 2], F32, tag="wy")
                nc.vector.tensor_scalar(wy[:, :, :, 1], d4[:, :, :, 1], 1.0, 0.5,
                                        op0=AX.mult, op1=AX.add)
                nc.vector.tensor_scalar(wy[:, :, :, 0], d4[:, :, :, 1], -1.0, 0.5,
                                        op0=AX.mult, op1=AX.add)
                wz = pool.tile([P, NB, GPC, 2], F32, tag="wz")
                nc.vector.tensor_scalar(wz[:, :, :, 1], d4[:, :, :, 2], 1.0, 0.5,
                                        op0=AX.mult, op1=AX.add)
                nc.vector.tensor_scalar(wz[:, :, :, 0], d4[:, :, :, 2], -1.0, 0.5,
                                        op0=AX.mult, op1=AX.add)
                # fold 3-axis mask into wz pair
                nc.vector.tensor_tensor(
                    wz[:], wz[:],
                    m3[:].rearrange("p b (g o) -> p b g o", o=1).to_broadcast([P, NB, GPC, 2]),
                    op=AX.mult)

                # wzy[dz,dy] = wz[dz]*wy[dy]; W[dz,dy,dx] = wzy*wx
                wzy = pool.tile([P, NB, GPC, 2, 2], F32, tag="wzy")
                nc.vector.tensor_tensor(
                    wzy[:],
                    wz[:].rearrange("p b g (z o) -> p b g z o", o=1).to_broadcast([P, NB, GPC, 2, 2]),
                    wy[:].rearrange("p b g (o y) -> p b g o y", o=1).to_broadcast([P, NB, GPC, 2, 2]),
                    op=AX.mult)
                W = pool.tile([P, NB, GPC, 8], F32, tag="W")
                W5 = W[:].rearrange("p b g (zy x) -> p b g zy x", x=2)
                nc.vector.tensor_tensor(
                    W5,
                    wzy[:].rearrange("p b g z (y o) -> p b g (z y) o", o=1).to_broadcast([P, NB, GPC, 4, 2]),
                    wx[:].rearrange("p b g (o x) -> p b g o x", o=1).to_broadcast([P, NB, GPC, 4, 2]),
                    op=AX.mult)

                # idx = cx + 65*cy + 4225*cz + base_g
                idxf = pool.tile([P, NB, GPC], F32, tag="idxf")
                nc.vector.tensor_scalar(idxf[:], c4[:, :, :, 1], 65.0, None, op0=AX.mult)
                nc.vector.tensor_tensor(idxf[:], idxf[:], c4[:, :, :, 0], op=AX.add)
                t2 = pool.tile([P, NB, GPC], F32, tag="t2")
                nc.vector.tensor_scalar(t2[:], c4[:, :, :, 2], 4225.0, None, op0=AX.mult)
                nc.vector.tensor_tensor(idxf[:], idxf[:], t2[:], op=AX.add)
                nc.vector.tensor_tensor(
                    idxf[:], idxf[:],
                    cb_sb[:].rearrange("p (o g) -> p o g", o=1).to_broadcast([P, NB, GPC]),
                    op=AX.add)
                idx = pool.tile([P, NB, GPC], I32, tag="idx")
                nc.vector.tensor_copy(idx[:], idxf[:])

                # gather: one 64B row per (point, grid)
                V = pool.tile([P, NB, GPC, 16], F32, tag="V")
                for b in range(NB):
                    for g in range(GPC):
                        nc.gpsimd.indirect_dma_start(
                            out=V[:, b, g, :],
                            out_offset=None,
                            in_=table[:],
                            in_offset=bass.IndirectOffsetOnAxis(
                                ap=idx[:, b, g:g + 1], axis=0))

                # weighted reduce: res[p, b, g, f] = sum_c V[p,b,g,f,c] * W[p,b,g,c]
                V5 = V[:].rearrange("p b g (f c) -> p b g f c", f=NF)
                nc.vector.tensor_tensor(
                    V5, V5,
                    W[:].rearrange("p b g (o c) -> p b g o c", o=1).to_broadcast([P, NB, GPC, NF, 8]),
                    op=AX.mult)
                res = pool.tile([P, NB, GPC, NF], F32, tag="res")
                nc.vector.tensor_reduce(res[:], V5, axis=mybir.AxisListType.X, op=AX.add)

                # out[n, g*2+f]: n = i*SUPER + b*128 + p
                nc.sync.dma_start(
                    out_v[i].rearrange("b p e -> p b e"),
                    res[:].rearrange("p b g f -> p b (g f)"))

            if n_iters == 1:
                body(0)
            else:
                tc.For_i_unrolled(0, n_iters, 1, body, max_unroll=unroll)
    nc.finalize()
    return nc


def prep_inputs(x, transformation_matrices, feature_grids):
    """Host-side layout prep. Returns in_maps list for 8 cores."""
    x = np.asarray(x, dtype=np.float32)
    M = np.asarray(transformation_matrices, dtype=np.float32)
    G = np.asarray(feature_grids, dtype=np.float32)
    n = x.shape[0]

    xt4 = np.empty((4, n), dtype=np.float32)
    xt4[:3] = x.T
    xt4[3] = 1.0

    # s = px + 0.5 = 31.5*(M x~)_i + 32.0
    Ms = M[:, :3, :] * np.float32(31.5)           # [64, 3, 4]
    Ms[:, :, 3] += np.float32(32.0)

    # corner-dilated table: R[g, cz, cy, cx, f*8+dz*4+dy*2+dx]
    Gp = np.zeros((N_GRIDS, NF, NPAD, NPAD, NPAD), dtype=np.float32)
    Gp[:, :, 1:65, 1:65, 1:65] = G
    R = np.empty((N_GRIDS, NC_CELL, NC_CELL, NC_CELL, 16), dtype=np.float32)
    for f in range(NF):
        for dz in range(2):
            for dy in range(2):
                for dx in range(2):
                    R[..., f * 8 + dz * 4 + dy * 2 + dx] = \
                        Gp[:, f, dz:dz + NC_CELL, dy:dy + NC_CELL, dx:dx + NC_CELL]
    R = R.reshape(N_GRIDS, CELLS, 16)

    cbase = np.broadcast_to(
        (np.arange(GPC, dtype=np.float32) * CELLS)[None, :], (P, GPC)).copy()

    in_maps = []
    for c in range(N_CORES):
        mp = np.empty((4, 24), dtype=np.float32)
        for g in range(GPC):
            for i in range(3):
                mp[:, g * 3 + i] = Ms[c * GPC + g, i, :]
        in_maps.append({
            "table": R[c * GPC:(c + 1) * GPC].reshape(T_ROWS, 16),
            "xt4": xt4,
            "mpack": mp,
            "cbase": cbase,
        })
    return in_maps


_NC_CACHE = {}


def kernel(x, transformation_matrices, feature_grids):
    key = "full"
    if key not in _NC_CACHE:
        _NC_CACHE[key] = build_kernel()
    nc = _NC_CACHE[key]
    in_maps = prep_inputs(x, transformation_matrices, feature_grids)
    res = run_bass_kernel_spmd(nc, in_maps, core_ids=list(range(N_CORES)))
    outs = [np.asarray(res.results[c]["out"]) for c in range(N_CORES)]
    return np.concatenate(outs, axis=1)

